# revision 11
# baseline (speedup 1.0000x reference)
"""Trainium2 Bass kernel for CRFHead (dense-Gaussian mean-field CRF).

Problem (hardcoded): B=2 images, 3x96x96, masks 96x96, N=9216 pixels,
10 mean-field iterations over the exact dense Gaussian kernel
K = exp(-0.5|f_i-f_j|^2), f = (x/60, y/60, rgb/5), symmetric-normalized.

Key structure exploited: with uint8 colors / sigma_rgb=5, K is ~99% tiny.
Pixels are sorted by the red channel; pairs with |dr| > T are dropped
(K <= exp(-T^2/50), negligible).  In sorted order each i-slot of 512/256
pixels only interacts with a contiguous, chunk-aligned j-window.

Per-core program (8 cores = 2 images x 4-way i-sharding, SPMD):
  - BUILD: one K=15 bf16 matmul per (slot, window-chunk) computes the
    pair exponents (features + both static |f|^2 terms ride the matmul);
    ACT exp writes the banded kernel matrix E into SBUF as fp16
    (j on partitions, i on the free dim).  ~140KB/partition, stays put.
  - deg / tvec / 10 iterations are then banded mat-vecs R = E^T w using
    M=1 matmuls (w chunk stationary, E chunks streamed), col-tiled 4x
    via tile_position.  Weights are fp16 hi+lo split pairs accumulated
    in one PSUM chain (restores f32-weight accuracy, which the
    near-chaotic mean-field dynamics require).
  - Between iterations only the tiny p = q/sqrt(deg) vector is
    exchanged (AllGather in each image's 4-core group).  Each core works
    in a local j-space = its own range +- one neighbor core; the two
    neighbor rows are fetched by an indirect DMA whose row indices are
    per-core input data, keeping the program core-uniform.

The j-window layout (chunk starts/widths per slot) is data-dependent; it
is computed on the host from the actual inputs and baked into the
compiled program (cached per window signature).
"""

import numpy as np
import ml_dtypes

B, C, H, W = 2, 3, 96, 96
N = H * W                      # 9216 pixels per image
N_CORES = 8
SHARDS = 4                     # cores per image
ROWS = N // SHARDS             # 2304 local pixels per core
TS = (512, 512, 512, 512, 256)             # i-slot sizes per core
OFF = (0, 512, 1024, 1536, 2048)           # i-slot offsets
PCOL = (0, 0, 0, 0, 512)                   # piece col block per slot
PPART = (0, 32, 64, 96, 0)                 # piece partition per slot
NSLOT = 5
LCHUNKS = 54                   # local j-space: 3 cores x 18 chunks
T_BAND = 25.0                  # red-channel band threshold
REFINE_ITERS = 10
RG = [[0, 1, 2, 3], [4, 5, 6, 7]]
KD = 15                        # matmul contraction rows

BF = ml_dtypes.bfloat16


def _bf(x):
    return np.asarray(x, dtype=BF).astype(np.float32)


def _split3(w):
    """3-way bf16 split of fp32 vector w (sum of parts ~= w)."""
    w = np.asarray(w, np.float32)
    w1 = np.asarray(w, BF)
    d1 = w - w1.astype(np.float32)
    w2 = np.asarray(d1, BF)
    w3 = np.asarray(d1 - w2.astype(np.float32), BF)
    return (w1.astype(np.float32), w2.astype(np.float32),
            w3.astype(np.float32))


def _host_prep(imgs, masks):
    """Mirror the reference's quantization exactly in numpy fp32."""
    imgs = np.asarray(imgs, np.float32)
    masks = np.asarray(masks, np.float32)
    MEAN = np.array([0.485, 0.456, 0.406], np.float32)[None, :, None, None]
    STD = np.array([0.229, 0.224, 0.225], np.float32)[None, :, None, None]
    x = (imgs * STD + MEAN).transpose(0, 2, 3, 1) * np.float32(255.0)
    x = np.floor(np.clip(x, 0.0, 255.0))
    m = np.floor(np.clip(masks * np.float32(255.0) / np.float32(0.7), 0.0, 255.0))
    return x, m


def _image_data(img_q, mask_q):
    """Sorted-order host arrays + per-(core,slot) windows for one image."""
    c = img_q.reshape(N, 3).astype(np.float32)
    perm = np.argsort(c[:, 0], kind="stable")
    rs = c[perm]

    ys, xs = np.meshgrid(np.arange(H, dtype=np.float32),
                         np.arange(W, dtype=np.float32), indexing="ij")
    xv = xs.reshape(N)[perm]
    yv = ys.reshape(N)[perm]

    U = mask_q / (mask_q.max() + np.float32(1e-8))
    U = np.clip(U, 1e-6, 1.0 - 1e-6).reshape(N).astype(np.float32)[perm]
    logitU = np.log(U / (np.float32(1.0) - U)).astype(np.float32)

    ax = (xv / np.float32(12.0)).astype(np.float32)
    ay = (yv / np.float32(12.0)).astype(np.float32)
    axh = _bf(ax); axl = _bf(ax - axh)
    ayh = _bf(ay); ayl = _bf(ay - ayh)
    r, g, b = _bf(rs[:, 0]), _bf(rs[:, 1]), _bf(rs[:, 2])
    ones = np.ones(N, np.float32)

    sqxy = xv * xv + yv * yv
    sqrgb = (rs * rs).sum(axis=1)
    wstat = (np.float32(25.0) * (-sqrgb / np.float32(50.0)
                                 - sqxy / np.float32(7200.0))).astype(np.float32)
    w1, w2, w3 = _split3(wstat)

    # j side (lhsT, partitions) and i side (rhs, free); PSUM[j,i]/25 =
    # f_j.f_i - 0.5|f_j|^2 - 0.5|f_i|^2 (modulo tiny axl*axl cross terms).
    j_rows = np.stack([r, g, b, axh, axh, axl, ayh, ayh, ayl,
                       ones, ones, ones, w1, w2, w3])
    i_rows = np.stack([r, g, b, axh, axl, axh, ayh, ayl, ayh,
                       w1, w2, w3, ones, ones, ones])

    # windows per (core, slot): chunk-aligned [rmin-T, rmax+T] in sorted r
    rsort = rs[:, 0]
    lo = np.empty((SHARDS, NSLOT), np.int64)
    hi = np.empty((SHARDS, NSLOT), np.int64)
    for g_ in range(SHARDS):
        for s in range(NSLOT):
            a = g_ * ROWS + OFF[s]
            bnd = a + TS[s]
            jlo = np.searchsorted(rsort, rsort[a] - T_BAND, side="left")
            jhi = np.searchsorted(rsort, rsort[bnd - 1] + T_BAND, side="right")
            lo[g_, s] = jlo // 128
            hi[g_, s] = -(-jhi // 128)
    return dict(perm=perm, U=U, logitU=logitU, j_rows=j_rows, i_rows=i_rows,
                lo=lo, hi=hi)


def _windows(per_image):
    """Uniform per-slot local-chunk windows (c0[s], W[s]) across cores+images."""
    c0 = np.full(NSLOT, 10 ** 9, np.int64)
    c1 = np.full(NSLOT, -10 ** 9, np.int64)
    for d in per_image:
        for g in range(SHARDS):
            # local chunk = global chunk - 18*(g-1)
            c0 = np.minimum(c0, d["lo"][g] - 18 * (g - 1))
            c1 = np.maximum(c1, d["hi"][g] - 18 * (g - 1))
    c0 = np.maximum(c0, 0)
    c1 = np.minimum(c1, LCHUNKS)
    w = c1 - c0
    assert (w > 0).all() and (c0 >= 0).all() and (c1 <= LCHUNKS).all()
    return tuple(int(v) for v in c0), tuple(int(v) for v in w)


def _core_inputs(data, g):
    """Per-core input tensors. Local j-space = global [2304(g-1), 2304(g+2))."""
    jf = np.zeros((KD, LCHUNKS * 128), np.float32)
    jf[12, :] = -60000.0  # dummy w1: exp -> 0 outside the global range
    glo = max(0, (g - 1) * ROWS)
    ghi = min(N, (g + 2) * ROWS)
    llo = glo - (g - 1) * ROWS
    jf[:, llo:llo + (ghi - glo)] = data["j_rows"][:, glo:ghi]

    iff = data["i_rows"][:, g * ROWS:(g + 1) * ROWS]

    vec = np.zeros((128, 2 * 768), np.float32)
    for s in range(NSLOT):
        sl = slice(g * ROWS + OFF[s], g * ROWS + OFF[s] + TS[s])
        vec[PPART[s], PCOL[s]:PCOL[s] + TS[s]] = data["logitU"][sl]
        vec[PPART[s], 768 + PCOL[s]:768 + PCOL[s] + TS[s]] = data["U"][sl]

    # rows of the padded gather buffer wgp[6]: row c+1 = core c; rows 0/5 = 0
    return {
        "jf": np.ascontiguousarray(jf.astype(BF)),
        "iff": np.ascontiguousarray(iff.astype(BF)),
        "vec": np.ascontiguousarray(vec),
        "idx": np.array([[g], [g + 2]], np.int32),
    }


def build_program(c0, wch):
    import concourse.bacc as bacc
    import concourse.mybir as mybir
    from concourse.tile import TileContext
    from concourse.bass import IndirectOffsetOnAxis

    f32 = mybir.dt.float32
    bf16 = mybir.dt.bfloat16
    fp16 = mybir.dt.float16
    i32 = mybir.dt.int32
    AF = mybir.ActivationFunctionType

    nc = bacc.Bacc(num_devices=N_CORES)

    jf_in = nc.dram_tensor("jf", [KD, LCHUNKS * 128], bf16, kind="ExternalInput")
    if_in = nc.dram_tensor("iff", [KD, ROWS], bf16, kind="ExternalInput")
    vec_in = nc.dram_tensor("vec", [128, 2 * 768], f32, kind="ExternalInput")
    idx_in = nc.dram_tensor("idx", [2, 1], i32, kind="ExternalInput")
    q_out_d = nc.dram_tensor("q_out", [1, ROWS], f32, kind="ExternalOutput")

    SCALE = float(np.float32(1.0) / np.float32(25.0))

    with TileContext(nc) as tc:
        with (
            tc.tile_pool(name="const", bufs=1) as cp,
            tc.tile_pool(name="vp", bufs=2) as vp,
            tc.tile_pool(name="psb", bufs=2, space="PSUM") as pb,
            tc.tile_pool(name="psm", bufs=1, space="PSUM") as pm,
            tc.tile_pool(name="dram", bufs=1, space="DRAM") as dp,
        ):
            # ---------------- persistent SBUF ----------------
            jf_sb = cp.tile([KD, LCHUNKS * 128], bf16, tag="jf")
            if_sb = cp.tile([KD, ROWS], bf16, tag="iff")
            vec_sb = cp.tile([128, 2 * 768], f32, tag="vec")
            logitU = vec_sb[:, 0:768]
            U_sb = vec_sb[:, 768:1536]
            idx_sb = cp.tile([2, 1], i32, tag="idx")
            E_sb = [cp.tile([128, wch[s] * TS[s]], fp16, tag=f"E{s}",
                            name=f"E{s}")
                    for s in range(NSLOT)]
            rsqd = cp.tile([128, 768], f32, tag="rsqd")
            Acoef = cp.tile([128, 768], f32, tag="Acoef")
            Ccoef = cp.tile([128, 768], f32, tag="Ccoef")
            # weight segments: [left, mid, right], fp16 [128, hl(2), 18]
            pseg = [cp.tile([128, 2, 18], fp16, tag=f"pseg{side}",
                            name=f"pseg{side}")
                    for side in range(3)]
            # indirect gather staging: row 0 = left neighbor, row 1 = right
            stage = cp.tile([2, 2 * ROWS], fp16, tag="stage")

            nc.sync.dma_start(out=jf_sb[:, :], in_=jf_in[:, :])
            nc.sync.dma_start(out=if_sb[:, :], in_=if_in[:, :])
            nc.sync.dma_start(out=vec_sb[:, :], in_=vec_in[:, :])
            nc.sync.dma_start(out=idx_sb[:, :], in_=idx_in[:, :])

            send = dp.tile([2, ROWS], fp16, tag="send")
            wgp = dp.tile([6, 2, ROWS], fp16, tag="wgp")
            nbr = dp.tile([2, 2 * ROWS], fp16, tag="nbr")

            # zero the padded gather rows (0 and 5) once
            zed = vp.tile([128, 36], fp16, tag="zed")
            nc.vector.memset(zed[:, :], 0.0)
            for row in (0, 5):
                nc.sync.dma_start(out=wgp[row, :, :], in_=zed[:, :])

            # ---------------- build banded E ----------------
            for s in range(NSLOT):
                per = 1536 // TS[s]          # psum chunks per ACT group
                ngrp = -(-wch[s] // per)
                for grp in range(ngrp):
                    ccs = range(grp * per, min(grp * per + per, wch[s]))
                    nk = len(ccs)
                    ps = pb.tile([128, 1536], f32, tag="bps")
                    for k, cc in enumerate(ccs):
                        lc = c0[s] + cc
                        nc.tensor.matmul(
                            ps[:, k * TS[s]:(k + 1) * TS[s]],
                            jf_sb[:, lc * 128:(lc + 1) * 128],
                            if_sb[:, OFF[s]:OFF[s] + TS[s]],
                            start=True, stop=True,
                        )
                    nc.scalar.activation(
                        E_sb[s][:, ccs.start * TS[s]:(ccs.start + nk) * TS[s]],
                        ps[:, 0:nk * TS[s]],
                        AF.Exp, scale=SCALE,
                    )

            # ---------------- mat-vec machinery ----------------
            ps_mv = pm.tile([128, 1024], f32, tag="mv")
            nc.vector.memset(ps_mv[:, :], 1.0)

            def matvec(split):
                """R = E^T w into ps_mv pieces; w from pseg (hi [+lo]).

                Emission is round-robin across slots so the 4 col-tiled
                strips actually run concurrently; within a slot, own (mid)
                chunks go first so the remote-neighbor fetch overlaps."""
                nhl = 2 if split else 1
                orders = [sorted(range(wch[s]),
                                 key=lambda cc: abs(c0[s] + cc - 26))
                          for s in range(NSLOT)]
                emitted = [0] * NSLOT
                for k in range(max(wch)):
                    for s in range(NSLOT):
                        if k >= wch[s]:
                            continue
                        cc = orders[s][k]
                        lc = c0[s] + cc
                        sg, col = pseg[lc // 18], lc % 18
                        out = ps_mv[PPART[s]:PPART[s] + 1,
                                    PCOL[s]:PCOL[s] + TS[s]]
                        nmm = wch[s] * nhl
                        for hl in range(nhl):
                            nc.tensor.matmul(
                                out,
                                sg[:, hl, col:col + 1],
                                E_sb[s][:, cc * TS[s]:(cc + 1) * TS[s]],
                                start=(emitted[s] == 0),
                                stop=(emitted[s] == nmm - 1),
                                tile_position=(0, PPART[s]),
                            )
                            emitted[s] += 1

            def send_pieces(phl):
                """merged hi/lo piece tile -> DRAM send rows, 5 DMAs."""
                for s in range(NSLOT):
                    nc.sync.dma_start(
                        out=send[:, OFF[s]:OFF[s] + TS[s]],
                        in_=phl[PPART[s]:PPART[s] + 1, :,
                                PCOL[s]:PCOL[s] + TS[s]],
                    )

            def distribute():
                """send -> AllGather -> own/mid direct + neighbors indirect."""
                nc.gpsimd.collective_compute(
                    "AllGather", mybir.AluOpType.bypass, replica_groups=RG,
                    ins=[send[:].opt()], outs=[wgp[1:5, :, :].opt()],
                )
                nc.sync.dma_start(
                    out=pseg[1][:, :, :],
                    in_=send[:, :].rearrange("h (c q) -> q h c", q=128),
                )
                nc.gpsimd.indirect_dma_start(
                    out=stage[:, :],
                    out_offset=None,
                    in_=wgp[:, :, :].rearrange("a h c -> a (h c)"),
                    in_offset=IndirectOffsetOnAxis(ap=idx_sb[:, 0:1], axis=0),
                )
                # SBUF->SBUF cannot repartition; bounce via DRAM
                nc.sync.dma_start(out=nbr[:, :], in_=stage[:, :])
                for side in (0, 2):
                    nc.sync.dma_start(
                        out=pseg[side][:, :, :],
                        in_=nbr[side // 2, :]
                        .rearrange("(h c q) -> q h c", h=2, q=128),
                    )

            def warm_pe(n):
                """Full-width dummy matmuls keep the HAM clock at 2.4GHz
                across the serial inter-pass window (M=1 mat-vecs alone
                leave the PE activity monitor throttled at 1.2GHz)."""
                for k in range(n):
                    ws = pb.tile([128, 1536], f32, tag="bps")
                    nc.tensor.matmul(
                        ws[:, 0:512], E_sb[0][:, 0:128], E_sb[0][:, 0:512],
                        start=True, stop=True, skip_group_check=True,
                    )

            def split_send(pf32):
                """f32 piece tile -> fp16 hi/lo pieces + send + distribute."""
                phl = vp.tile([128, 2, 768], fp16, tag="phl")
                nc.vector.tensor_copy(phl[:, 0, :], pf32[:, :])
                nc.vector.tensor_sub(phl[:, 1, :], pf32[:, :], phl[:, 0, :])
                send_pieces(phl)
                distribute()
                warm_pe(24)

            # ---------------- deg -> rsqd, A ----------------
            for side in range(3):
                nc.vector.memset(pseg[side][:, 0, :], 1.0)
                nc.vector.memset(pseg[side][:, 1, :], 0.0)
            matvec(split=False)
            lnd = vp.tile([128, 768], f32, tag="t")
            nc.scalar.activation(lnd[:, :], ps_mv[:, 0:768], AF.Ln)
            nc.scalar.activation(rsqd[:, :], lnd[:, :], AF.Exp, scale=-0.5)
            nc.vector.tensor_scalar_mul(Acoef[:, :], rsqd[:, :], 10.0)
            split_send(rsqd)

            # ---------------- tvec -> C ----------------
            matvec(split=True)
            t1 = vp.tile([128, 768], f32, tag="t")
            nc.vector.tensor_mul(t1[:, :], rsqd[:, :], ps_mv[:, 0:768])
            t2 = vp.tile([128, 768], f32, tag="z")
            nc.vector.tensor_scalar_mul(t2[:, :], t1[:, :], -5.0)
            nc.vector.tensor_add(Ccoef[:, :], logitU[:, :], t2[:, :])

            # ---------------- p0 = U * rsqd ----------------
            p0 = vp.tile([128, 768], f32, tag="p")
            nc.vector.tensor_mul(p0[:, :], U_sb[:, :], rsqd[:, :])
            split_send(p0)

            # ---------------- 10 mean-field iterations ----------------
            q = None
            for it in range(REFINE_ITERS):
                matvec(split=True)
                t = vp.tile([128, 768], f32, tag="t")
                nc.vector.tensor_mul(t[:, :], Acoef[:, :], ps_mv[:, 0:768])
                z = vp.tile([128, 768], f32, tag="z")
                nc.vector.tensor_add(z[:, :], Ccoef[:, :], t[:, :])
                q = vp.tile([128, 768], f32, tag="q")
                nc.scalar.activation(q[:, :], z[:, :], AF.Sigmoid)
                if it < REFINE_ITERS - 1:
                    p = vp.tile([128, 768], f32, tag="p")
                    nc.vector.tensor_mul(p[:, :], q[:, :], rsqd[:, :])
                    split_send(p)

            for s in range(NSLOT):
                nc.sync.dma_start(
                    out=q_out_d[0, OFF[s]:OFF[s] + TS[s]],
                    in_=q[PPART[s]:PPART[s] + 1, PCOL[s]:PCOL[s] + TS[s]],
                )

    nc.compile()
    _fix_act_table_loads(nc, mybir)
    return nc


def _fix_act_table_loads(nc, mybir):
    """Point Exp/Ln loads at one shared set; drop same-set reloads."""
    from concourse.hw_specs import get_activation_tables
    AF = mybir.ActivationFunctionType
    tables = list(get_activation_tables(nc.m.arch).items())
    exp_ln = None
    sig = None
    for idx, (_, funcs) in enumerate(tables):
        if exp_ln is None and {AF.Exp, AF.Ln} <= funcs:
            exp_ln = idx
        if sig is None and AF.Sigmoid in funcs:
            sig = idx
    for blk in nc.m.functions[0].blocks:
        il = blk.instructions
        cur = None
        drop = []
        pending = None
        for ins in il:
            tn = type(ins).__name__
            if tn == "InstLoadActFuncSet":
                sync = getattr(ins, "sync_info", None)
                if sync is not None and (sync.on_wait or sync.on_update):
                    cur = None  # unknown state; keep following loads
                    continue
                pending = ins
            elif tn == "InstActivation" and pending is not None:
                f = pending.act_func_set_id
                if ins.func in (AF.Exp, AF.Ln) and exp_ln is not None:
                    f = exp_ln
                elif ins.func == AF.Sigmoid and sig is not None:
                    f = sig
                if f == cur:
                    drop.append(pending)
                else:
                    pending.act_func_set_id = f
                    cur = f
                pending = None
        for ins in drop:
            il.remove(ins)


_NC_CACHE = {}


def make_in_maps(imgs, masks):
    x, m = _host_prep(imgs, masks)
    per_image = [_image_data(x[b], m[b]) for b in range(B)]
    c0, wch = _windows(per_image)
    in_maps = []
    for k in range(N_CORES):
        b, g = divmod(k, SHARDS)
        in_maps.append(_core_inputs(per_image[b], g))
    return in_maps, per_image, c0, wch


def assemble(results, per_image):
    out = np.empty((B, N), np.float32)
    for k in range(N_CORES):
        b, g = divmod(k, SHARDS)
        q = np.asarray(results[k]["q_out"], np.float32).reshape(ROWS)
        perm = per_image[b]["perm"]
        out[b, perm[g * ROWS:(g + 1) * ROWS]] = q
    return out.reshape(B, H, W)


def kernel(imgs, masks):
    from concourse.bass_utils import run_bass_kernel_spmd

    in_maps, per_image, c0, wch = make_in_maps(imgs, masks)
    key = (c0, wch)
    if key not in _NC_CACHE:
        _NC_CACHE[key] = build_program(c0, wch)
    res = run_bass_kernel_spmd(_NC_CACHE[key], in_maps, list(range(N_CORES)))
    return assemble(res.results, per_image)


# revision 14
# speedup vs baseline: 1.1363x; 1.1363x over previous
"""Trainium2 Bass kernel for CRFHead (dense-Gaussian mean-field CRF).

Problem (hardcoded): B=2 images, 3x96x96, masks 96x96, N=9216 pixels,
10 mean-field iterations over the exact dense Gaussian kernel
K = exp(-0.5|f_i-f_j|^2), f = (x/60, y/60, rgb/5), symmetric-normalized.

Key structure exploited: with uint8 colors / sigma_rgb=5, K is ~99% tiny.
Pixels are sorted by the red channel; pairs with |dr| > T are dropped
(K <= exp(-T^2/50), negligible).  In sorted order each i-slot of 512/256
pixels only interacts with a contiguous, chunk-aligned j-window.

Per-core program (8 cores = 2 images x 4-way i-sharding, SPMD):
  - BUILD: one K=15 bf16 matmul per (slot, window-chunk) computes the
    pair exponents (features + both static |f|^2 terms ride the matmul);
    ACT exp writes the banded kernel matrix E into SBUF as fp16
    (j on partitions, i on the free dim).  ~140KB/partition, stays put.
  - deg / tvec / 10 iterations are then banded mat-vecs R = E^T w using
    M=1 matmuls (w chunk stationary, E chunks streamed), col-tiled 4x
    via tile_position.  Weights are fp16 hi+lo split pairs accumulated
    in one PSUM chain (restores f32-weight accuracy, which the
    near-chaotic mean-field dynamics require).
  - Between iterations only the tiny p = q/sqrt(deg) vector is
    exchanged (AllGather in each image's 4-core group).  Each core works
    in a local j-space = its own range +- one neighbor core; the two
    neighbor rows are fetched by an indirect DMA whose row indices are
    per-core input data, keeping the program core-uniform.

The j-window layout (chunk starts/widths per slot) is data-dependent; it
is computed on the host from the actual inputs and baked into the
compiled program (cached per window signature).
"""

import numpy as np
import ml_dtypes

B, C, H, W = 2, 3, 96, 96
N = H * W                      # 9216 pixels per image
N_CORES = 8
SHARDS = 4                     # cores per image
ROWS = N // SHARDS             # 2304 local pixels per core
TS = (512, 512, 512, 512, 256)             # i-slot sizes per core
OFF = (0, 512, 1024, 1536, 2048)           # i-slot offsets
PCOL = (0, 0, 0, 0, 512)                   # piece col block per slot
PPART = (0, 32, 64, 96, 0)                 # piece partition per slot
NSLOT = 5
LCHUNKS = 54                   # local j-space: 3 cores x 18 chunks
T_BAND = 25.0                  # red-channel band threshold
REFINE_ITERS = 10
RG = [[0, 1, 2, 3], [4, 5, 6, 7]]
KD = 15                        # matmul contraction rows

BF = ml_dtypes.bfloat16


def _bf(x):
    return np.asarray(x, dtype=BF).astype(np.float32)


def _split3(w):
    """3-way bf16 split of fp32 vector w (sum of parts ~= w)."""
    w = np.asarray(w, np.float32)
    w1 = np.asarray(w, BF)
    d1 = w - w1.astype(np.float32)
    w2 = np.asarray(d1, BF)
    w3 = np.asarray(d1 - w2.astype(np.float32), BF)
    return (w1.astype(np.float32), w2.astype(np.float32),
            w3.astype(np.float32))


def _host_prep(imgs, masks):
    """Mirror the reference's quantization exactly in numpy fp32."""
    imgs = np.asarray(imgs, np.float32)
    masks = np.asarray(masks, np.float32)
    MEAN = np.array([0.485, 0.456, 0.406], np.float32)[None, :, None, None]
    STD = np.array([0.229, 0.224, 0.225], np.float32)[None, :, None, None]
    x = (imgs * STD + MEAN).transpose(0, 2, 3, 1) * np.float32(255.0)
    x = np.floor(np.clip(x, 0.0, 255.0))
    m = np.floor(np.clip(masks * np.float32(255.0) / np.float32(0.7), 0.0, 255.0))
    return x, m


def _image_data(img_q, mask_q):
    """Sorted-order host arrays + per-(core,slot) windows for one image."""
    c = img_q.reshape(N, 3).astype(np.float32)
    perm = np.argsort(c[:, 0], kind="stable")
    rs = c[perm]

    ys, xs = np.meshgrid(np.arange(H, dtype=np.float32),
                         np.arange(W, dtype=np.float32), indexing="ij")
    xv = xs.reshape(N)[perm]
    yv = ys.reshape(N)[perm]

    U = mask_q / (mask_q.max() + np.float32(1e-8))
    U = np.clip(U, 1e-6, 1.0 - 1e-6).reshape(N).astype(np.float32)[perm]
    logitU = np.log(U / (np.float32(1.0) - U)).astype(np.float32)

    ax = (xv / np.float32(12.0)).astype(np.float32)
    ay = (yv / np.float32(12.0)).astype(np.float32)
    axh = _bf(ax); axl = _bf(ax - axh)
    ayh = _bf(ay); ayl = _bf(ay - ayh)
    r, g, b = _bf(rs[:, 0]), _bf(rs[:, 1]), _bf(rs[:, 2])
    ones = np.ones(N, np.float32)

    sqxy = xv * xv + yv * yv
    sqrgb = (rs * rs).sum(axis=1)
    wstat = (np.float32(25.0) * (-sqrgb / np.float32(50.0)
                                 - sqxy / np.float32(7200.0))).astype(np.float32)
    w1, w2, w3 = _split3(wstat)

    # j side (lhsT, partitions) and i side (rhs, free); PSUM[j,i]/25 =
    # f_j.f_i - 0.5|f_j|^2 - 0.5|f_i|^2 (modulo tiny axl*axl cross terms).
    j_rows = np.stack([r, g, b, axh, axh, axl, ayh, ayh, ayl,
                       ones, ones, ones, w1, w2, w3])
    i_rows = np.stack([r, g, b, axh, axl, axh, ayh, ayl, ayh,
                       w1, w2, w3, ones, ones, ones])

    # windows per (core, slot): chunk-aligned [rmin-T, rmax+T] in sorted r
    rsort = rs[:, 0]
    lo = np.empty((SHARDS, NSLOT), np.int64)
    hi = np.empty((SHARDS, NSLOT), np.int64)
    for g_ in range(SHARDS):
        for s in range(NSLOT):
            a = g_ * ROWS + OFF[s]
            bnd = a + TS[s]
            jlo = np.searchsorted(rsort, rsort[a] - T_BAND, side="left")
            jhi = np.searchsorted(rsort, rsort[bnd - 1] + T_BAND, side="right")
            lo[g_, s] = jlo // 128
            hi[g_, s] = -(-jhi // 128)
    return dict(perm=perm, U=U, logitU=logitU, j_rows=j_rows, i_rows=i_rows,
                lo=lo, hi=hi)


def _windows(per_image):
    """Uniform per-slot local-chunk windows (c0[s], W[s]) across cores+images."""
    c0 = np.full(NSLOT, 10 ** 9, np.int64)
    c1 = np.full(NSLOT, -10 ** 9, np.int64)
    for d in per_image:
        for g in range(SHARDS):
            # local chunk = global chunk - 18*(g-1)
            c0 = np.minimum(c0, d["lo"][g] - 18 * (g - 1))
            c1 = np.maximum(c1, d["hi"][g] - 18 * (g - 1))
    c0 = np.maximum(c0, 0)
    c1 = np.minimum(c1, LCHUNKS)
    w = c1 - c0
    assert (w > 0).all() and (c0 >= 0).all() and (c1 <= LCHUNKS).all()
    return tuple(int(v) for v in c0), tuple(int(v) for v in w)


def _core_inputs(data, g):
    """Per-core input tensors. Local j-space = global [2304(g-1), 2304(g+2))."""
    jf = np.zeros((KD, LCHUNKS * 128), np.float32)
    jf[12, :] = -60000.0  # dummy w1: exp -> 0 outside the global range
    glo = max(0, (g - 1) * ROWS)
    ghi = min(N, (g + 2) * ROWS)
    llo = glo - (g - 1) * ROWS
    jf[:, llo:llo + (ghi - glo)] = data["j_rows"][:, glo:ghi]

    iff = data["i_rows"][:, g * ROWS:(g + 1) * ROWS]

    vec = np.zeros((128, 2 * 768), np.float32)
    for s in range(NSLOT):
        sl = slice(g * ROWS + OFF[s], g * ROWS + OFF[s] + TS[s])
        vec[PPART[s], PCOL[s]:PCOL[s] + TS[s]] = data["logitU"][sl]
        vec[PPART[s], 768 + PCOL[s]:768 + PCOL[s] + TS[s]] = data["U"][sl]

    # rows of the padded gather buffer wgp[6]: row c+1 = core c; rows 0/5 = 0
    return {
        "jf": np.ascontiguousarray(jf.astype(BF)),
        "iff": np.ascontiguousarray(iff.astype(BF)),
        "vec": np.ascontiguousarray(vec),
        "idx": np.array([[g], [g + 2]], np.int32),
    }


def build_program(c0, wch):
    import concourse.bacc as bacc
    import concourse.mybir as mybir
    from concourse.tile import TileContext
    from concourse.bass import IndirectOffsetOnAxis

    f32 = mybir.dt.float32
    bf16 = mybir.dt.bfloat16
    fp16 = mybir.dt.float16
    i32 = mybir.dt.int32
    AF = mybir.ActivationFunctionType

    nc = bacc.Bacc(num_devices=N_CORES)

    jf_in = nc.dram_tensor("jf", [KD, LCHUNKS * 128], bf16, kind="ExternalInput")
    if_in = nc.dram_tensor("iff", [KD, ROWS], bf16, kind="ExternalInput")
    vec_in = nc.dram_tensor("vec", [128, 2 * 768], f32, kind="ExternalInput")
    idx_in = nc.dram_tensor("idx", [2, 1], i32, kind="ExternalInput")
    q_out_d = nc.dram_tensor("q_out", [1, ROWS], f32, kind="ExternalOutput")

    SCALE = float(np.float32(1.0) / np.float32(25.0))

    with TileContext(nc) as tc:
        with (
            tc.tile_pool(name="const", bufs=1) as cp,
            tc.tile_pool(name="vp", bufs=2) as vp,
            tc.tile_pool(name="psb", bufs=2, space="PSUM") as pb,
            tc.tile_pool(name="psm", bufs=1, space="PSUM") as pm,
            tc.tile_pool(name="dram", bufs=1, space="DRAM") as dp,
        ):
            # ---------------- persistent SBUF ----------------
            jf_sb = cp.tile([KD, LCHUNKS * 128], bf16, tag="jf")
            if_sb = cp.tile([KD, ROWS], bf16, tag="iff")
            vec_sb = cp.tile([128, 2 * 768], f32, tag="vec")
            logitU = vec_sb[:, 0:768]
            U_sb = vec_sb[:, 768:1536]
            idx_sb = cp.tile([2, 1], i32, tag="idx")
            E_sb = [cp.tile([128, wch[s] * TS[s]], fp16, tag=f"E{s}",
                            name=f"E{s}")
                    for s in range(NSLOT)]
            rsqd = cp.tile([128, 768], f32, tag="rsqd")
            Acoef = cp.tile([128, 768], f32, tag="Acoef")
            Ccoef = cp.tile([128, 768], f32, tag="Ccoef")
            # weight segments: [left, mid, right], fp16 [128, hl(2), 18]
            pseg = [cp.tile([128, 2, 18], fp16, tag=f"pseg{side}",
                            name=f"pseg{side}")
                    for side in range(3)]
            # indirect gather staging: row 0 = left neighbor, row 1 = right
            stage = cp.tile([2, 2 * ROWS], fp16, tag="stage")
            ccnk = cp.tile([128, ROWS], fp16, tag="ccnk")
            ones2 = cp.tile([128, 1], fp16, tag="ones2")

            nc.sync.dma_start(out=jf_sb[:, :], in_=jf_in[:, :])
            nc.sync.dma_start(out=if_sb[:, :], in_=if_in[:, :])
            nc.sync.dma_start(out=vec_sb[:, :], in_=vec_in[:, :])
            nc.sync.dma_start(out=idx_sb[:, :], in_=idx_in[:, :])

            send = dp.tile([2, ROWS], fp16, tag="send")
            cdr = dp.tile([2, ROWS], fp16, tag="cdr")
            wgp = dp.tile([6, 2, ROWS], fp16, tag="wgp")
            nbr = dp.tile([2, 2 * ROWS], fp16, tag="nbr")

            # zero the padded gather rows (0 and 5) once
            zed = vp.tile([128, 36], fp16, tag="zed")
            nc.vector.memset(zed[:, :], 0.0)
            nc.vector.memset(ones2[:, :], 0.0)
            nc.vector.memset(ones2[0:2, :], 1.0)
            for row in (0, 5):
                nc.sync.dma_start(out=wgp[row, :, :], in_=zed[:, :])

            # ---------------- build banded E ----------------
            for s in range(NSLOT):
                per = 1536 // TS[s]          # psum chunks per ACT group
                ngrp = -(-wch[s] // per)
                for grp in range(ngrp):
                    ccs = range(grp * per, min(grp * per + per, wch[s]))
                    nk = len(ccs)
                    ps = pb.tile([128, 1536], f32, tag="bps")
                    for k, cc in enumerate(ccs):
                        lc = c0[s] + cc
                        nc.tensor.matmul(
                            ps[:, k * TS[s]:(k + 1) * TS[s]],
                            jf_sb[:, lc * 128:(lc + 1) * 128],
                            if_sb[:, OFF[s]:OFF[s] + TS[s]],
                            start=True, stop=True,
                        )
                    nc.scalar.activation(
                        E_sb[s][:, ccs.start * TS[s]:(ccs.start + nk) * TS[s]],
                        ps[:, 0:nk * TS[s]],
                        AF.Exp, scale=SCALE,
                    )

            # ---------------- mat-vec machinery ----------------
            ps_mv = pm.tile([128, 1024], f32, tag="mv")
            nc.vector.memset(ps_mv[:, :], 1.0)

            def matvec(split, bias=False):
                """R = E^T w into ps_mv pieces; w from pseg (hi [+lo]).

                Emission is round-robin across slots so the 4 col-tiled
                strips actually run concurrently; within a slot, own (mid)
                chunks go first so the remote-neighbor fetch overlaps.
                bias=True appends a constant chunk accumulating C/A."""
                nhl = 2 if split else 1
                orders = [sorted(range(wch[s]),
                                 key=lambda cc: abs(c0[s] + cc - 26))
                          for s in range(NSLOT)]
                emitted = [0] * NSLOT
                for s in range(NSLOT):
                    if bias:
                        nc.tensor.matmul(
                            ps_mv[PPART[s]:PPART[s] + 1,
                                  PCOL[s]:PCOL[s] + TS[s]],
                            ones2[:, 0:1],
                            ccnk[:, OFF[s]:OFF[s] + TS[s]],
                            start=True, stop=False,
                            tile_position=(0, PPART[s]),
                        )
                for k in range(max(wch)):
                    for s in range(NSLOT):
                        if k >= wch[s]:
                            continue
                        cc = orders[s][k]
                        lc = c0[s] + cc
                        sg, col = pseg[lc // 18], lc % 18
                        out = ps_mv[PPART[s]:PPART[s] + 1,
                                    PCOL[s]:PCOL[s] + TS[s]]
                        nmm = wch[s] * nhl
                        for hl in range(nhl):
                            nc.tensor.matmul(
                                out,
                                sg[:, hl, col:col + 1],
                                E_sb[s][:, cc * TS[s]:(cc + 1) * TS[s]],
                                start=(not bias) and (emitted[s] == 0),
                                stop=(emitted[s] == nmm - 1),
                                tile_position=(0, PPART[s]),
                            )
                            emitted[s] += 1

            def send_pieces(phl):
                """merged hi/lo piece tile -> DRAM send rows, 2 DMAs."""
                nc.sync.dma_start(
                    out=send[:, 0:2048].rearrange("h (s w) -> s h w", s=4),
                    in_=phl[0:128:32, :, 0:512],
                )
                nc.scalar.dma_start(
                    out=send[:, 2048:2304],
                    in_=phl[0:1, :, 512:768],
                )

            def distribute():
                """send -> AllGather -> own/mid direct + neighbors indirect."""
                nc.gpsimd.collective_compute(
                    "AllGather", mybir.AluOpType.bypass, replica_groups=RG,
                    ins=[send[:].opt()], outs=[wgp[1:5, :, :].opt()],
                )
                nc.scalar.dma_start(
                    out=pseg[1][:, :, :],
                    in_=send[:, :].rearrange("h (c q) -> q h c", q=128),
                )
                nc.gpsimd.indirect_dma_start(
                    out=stage[:, :],
                    out_offset=None,
                    in_=wgp[:, :, :].rearrange("a h c -> a (h c)"),
                    in_offset=IndirectOffsetOnAxis(ap=idx_sb[:, 0:1], axis=0),
                )
                # SBUF->SBUF cannot repartition; bounce via DRAM
                nc.sync.dma_start(out=nbr[:, :], in_=stage[:, :])
                for side, eng in ((0, nc.sync), (2, nc.scalar)):
                    eng.dma_start(
                        out=pseg[side][:, :, :],
                        in_=nbr[side // 2, :]
                        .rearrange("(h c q) -> q h c", h=2, q=128),
                    )

            def warm_pe(n, dep):
                """Full-width dummy matmuls anchored on `dep` keep the HAM
                clock at 2.4GHz across the serial inter-pass window (M=1
                mat-vecs alone leave the PE activity monitor throttled)."""
                for k in range(n):
                    ws = pb.tile([128, 1536], f32, tag="bps")
                    nc.tensor.matmul(
                        ws[:, 0:512], dep[:, 0:128], E_sb[0][:, 0:512],
                        start=True, stop=True, skip_group_check=True,
                    )

            def split_send(pf32):
                """f32 piece tile -> fp16 hi/lo pieces + send + distribute."""
                phl = vp.tile([128, 2, 768], fp16, tag="phl")
                nc.vector.tensor_copy(phl[:, 0, :], pf32[:, :])
                nc.vector.tensor_sub(phl[:, 1, :], pf32[:, :], phl[:, 0, :])
                send_pieces(phl)
                distribute()
                warm_pe(16, phl[:, 0, :])

            # ---------------- deg -> rsqd, A ----------------
            for side in range(3):
                nc.vector.memset(pseg[side][:, 0, :], 1.0)
                nc.vector.memset(pseg[side][:, 1, :], 0.0)
            matvec(split=False)
            lnd = vp.tile([128, 768], f32, tag="t")
            nc.scalar.activation(lnd[:, :], ps_mv[:, 0:768], AF.Ln)
            nc.scalar.activation(rsqd[:, :], lnd[:, :], AF.Exp, scale=-0.5)
            nc.vector.tensor_scalar_mul(Acoef[:, :], rsqd[:, :], 10.0)
            split_send(rsqd)

            # ---------------- tvec -> C ----------------
            matvec(split=True)
            t1 = vp.tile([128, 768], f32, tag="t")
            nc.vector.tensor_mul(t1[:, :], rsqd[:, :], ps_mv[:, 0:768])
            t2 = vp.tile([128, 768], f32, tag="q")
            nc.vector.tensor_scalar_mul(t2[:, :], t1[:, :], -5.0)
            nc.vector.tensor_add(Ccoef[:, :], logitU[:, :], t2[:, :])

            # ---------------- ccnk: C/A as a constant bias chunk ----
            ca = vp.tile([128, 768], f32, tag="t")
            rA = vp.tile([128, 768], f32, tag="q")
            nc.vector.reciprocal(rA[:, :], Acoef[:, :])
            nc.vector.tensor_mul(ca[:, :], Ccoef[:, :], rA[:, :])
            cah = vp.tile([128, 2, 768], fp16, tag="phl")
            nc.vector.tensor_copy(cah[:, 0, :], ca[:, :])
            nc.vector.tensor_sub(cah[:, 1, :], ca[:, :], cah[:, 0, :])
            nc.vector.memset(ccnk[:, :], 0.0)
            nc.sync.dma_start(
                out=cdr[:, 0:2048].rearrange("h (s w) -> s h w", s=4),
                in_=cah[0:128:32, :, 0:512],
            )
            nc.sync.dma_start(out=cdr[:, 2048:2304], in_=cah[0:1, :, 512:768])
            nc.sync.dma_start(out=ccnk[0:2, :], in_=cdr[:, :])

            # ---------------- p0 = U * rsqd ----------------
            p0 = vp.tile([128, 768], f32, tag="p")
            nc.vector.tensor_mul(p0[:, :], U_sb[:, :], rsqd[:, :])
            split_send(p0)

            # ---------------- 10 mean-field iterations ----------------
            q = None
            for it in range(REFINE_ITERS):
                matvec(split=True, bias=True)
                t = vp.tile([128, 768], f32, tag="t")
                nc.vector.tensor_mul(t[:, :], Acoef[:, :], ps_mv[:, 0:768])
                q = vp.tile([128, 768], f32, tag="q")
                nc.scalar.activation(q[:, :], t[:, :], AF.Sigmoid)
                if it < REFINE_ITERS - 1:
                    p = vp.tile([128, 768], f32, tag="p")
                    nc.vector.tensor_mul(p[:, :], q[:, :], rsqd[:, :])
                    split_send(p)

            for s in range(NSLOT):
                nc.sync.dma_start(
                    out=q_out_d[0, OFF[s]:OFF[s] + TS[s]],
                    in_=q[PPART[s]:PPART[s] + 1, PCOL[s]:PCOL[s] + TS[s]],
                )

    nc.compile()
    _fix_act_table_loads(nc, mybir)
    return nc


def _fix_act_table_loads(nc, mybir):
    """Point Exp/Ln loads at one shared set; drop same-set reloads."""
    from concourse.hw_specs import get_activation_tables
    AF = mybir.ActivationFunctionType
    tables = list(get_activation_tables(nc.m.arch).items())
    exp_ln = None
    sig = None
    for idx, (_, funcs) in enumerate(tables):
        if exp_ln is None and {AF.Exp, AF.Ln} <= funcs:
            exp_ln = idx
        if sig is None and AF.Sigmoid in funcs:
            sig = idx
    for blk in nc.m.functions[0].blocks:
        il = blk.instructions
        cur = None
        drop = []
        pending = None
        for ins in il:
            tn = type(ins).__name__
            if tn == "InstLoadActFuncSet":
                sync = getattr(ins, "sync_info", None)
                if sync is not None and (sync.on_wait or sync.on_update):
                    cur = None  # unknown state; keep following loads
                    continue
                pending = ins
            elif tn == "InstActivation" and pending is not None:
                f = pending.act_func_set_id
                if ins.func in (AF.Exp, AF.Ln) and exp_ln is not None:
                    f = exp_ln
                elif ins.func == AF.Sigmoid and sig is not None:
                    f = sig
                if f == cur:
                    drop.append(pending)
                else:
                    pending.act_func_set_id = f
                    cur = f
                pending = None
        for ins in drop:
            il.remove(ins)


_NC_CACHE = {}


def make_in_maps(imgs, masks):
    x, m = _host_prep(imgs, masks)
    per_image = [_image_data(x[b], m[b]) for b in range(B)]
    c0, wch = _windows(per_image)
    in_maps = []
    for k in range(N_CORES):
        b, g = divmod(k, SHARDS)
        in_maps.append(_core_inputs(per_image[b], g))
    return in_maps, per_image, c0, wch


def assemble(results, per_image):
    out = np.empty((B, N), np.float32)
    for k in range(N_CORES):
        b, g = divmod(k, SHARDS)
        q = np.asarray(results[k]["q_out"], np.float32).reshape(ROWS)
        perm = per_image[b]["perm"]
        out[b, perm[g * ROWS:(g + 1) * ROWS]] = q
    return out.reshape(B, H, W)


def kernel(imgs, masks):
    from concourse.bass_utils import run_bass_kernel_spmd

    in_maps, per_image, c0, wch = make_in_maps(imgs, masks)
    key = (c0, wch)
    if key not in _NC_CACHE:
        _NC_CACHE[key] = build_program(c0, wch)
    res = run_bass_kernel_spmd(_NC_CACHE[key], in_maps, list(range(N_CORES)))
    return assemble(res.results, per_image)
